# revision 1
# baseline (speedup 1.0000x reference)
"""GCN block (3x [linear+relu -> COO SpMM]) on 8 trn2 NeuronCores.

B=8, N=10000, D=64, E=160000. Batch-parallel: one batch per core, identical
SPMD program (adjacency and weights shared), no collectives.

Device algorithm per core / per layer:
  dense:  H'[n,f] = relu(sum_d H_fm[d,n] * W.T[d,f]) via PE matmuls
          (lhsT = feature-major H slice, zero transposes anywhere),
          row-major H' streamed to DRAM.
  spmm:   edges host-sorted by output row; dma_gather fetches the 256B
          source rows for 8192 edges per call (3 SWDGE queues in parallel -
          the gather is descriptor-rate-bound); PE computes the scatter-add
          as a segmented matmul  psum[64, rows] += Mg[128e,64].T @ S[128e,R]
          where the host-built S holds adj_vals at (edge, local_row).
          PSUM accumulates per 512-row group (opened by a zeroing matmul),
          then DVE copies the group back to feature-major SBUF for the next
          layer (or out to DRAM on the last layer; host transposes).
"""
import sys

sys.path.insert(0, "/opt/trn_rl_repo")

import numpy as np

B, N, D, E = 8, 10000, 64, 160000
GROUP = 512
NG = (N + GROUP - 1) // GROUP
CHUNK = 128
SC_MAX = 8192  # edges per dma_gather call
NQUEUES = 3    # SWDGE queues (4 is faster but was flaky in testing)

_CACHE = {}


def _preprocess(adj_vals, adj_rows, adj_cols):
    rows = adj_rows.astype(np.int64)
    cols = adj_cols.astype(np.int64)
    vals = adj_vals.astype(np.float32)

    # every output row needs >= 1 edge so every group exists and every PSUM
    # column is PE-written before the DVE copy
    deg = np.bincount(rows, minlength=N)
    missing = np.nonzero(deg == 0)[0]
    if len(missing):
        rows = np.concatenate([rows, missing])
        cols = np.concatenate([cols, np.zeros(len(missing), np.int64)])
        vals = np.concatenate([vals, np.zeros(len(missing), np.float32)])

    order = np.argsort(rows, kind="stable")
    rows, cols, vals = rows[order], cols[order], vals[order]

    cols_p = []
    chunk_meta = []  # (group, r_lo, R, s_off, first_of_group, last_of_group)
    s_blocks = []
    s_off = 0
    bounds = np.searchsorted(rows, np.arange(0, NG + 1) * GROUP)
    for g in range(NG):
        lo, hi = g * GROUP, min((g + 1) * GROUP, N)
        a, b = bounds[g], bounds[g + 1]
        r, c, v = rows[a:b], cols[a:b], vals[a:b]
        pad = (-len(r)) % CHUNK
        if pad:
            r = np.concatenate([r, np.full(pad, hi - 1, np.int64)])
            c = np.concatenate([c, np.zeros(pad, np.int64)])
            v = np.concatenate([v, np.zeros(pad, np.float32)])
        cols_p.append(c)
        nch = len(r) // CHUNK
        for j in range(nch):
            rl = (r[j * CHUNK : (j + 1) * CHUNK] - lo).astype(np.int64)
            vv = v[j * CHUNK : (j + 1) * CHUNK]
            r_lo = int(rl.min())
            R = int(rl.max()) - r_lo + 1
            S = np.zeros((CHUNK, R), np.float32)
            S[np.arange(CHUNK), rl - r_lo] = vv
            s_blocks.append(S)
            chunk_meta.append((g, r_lo, R, s_off, j == 0, j == nch - 1))
            s_off += R

    cols_all = np.concatenate(cols_p)
    e_pad = len(cols_all)
    S_packed = np.ascontiguousarray(np.concatenate(s_blocks, axis=1))

    # dma_gather index layout: index i lives at partition i%16, column i//16,
    # replicated across the 8 Q7 cores' 16-partition groups
    idx16 = cols_all.astype(np.int16).reshape(e_pad // 16, 16).T
    idx_layout = np.ascontiguousarray(np.tile(idx16, (8, 1)))

    superchunks = []
    e0 = 0
    while e0 < e_pad:
        n = min(SC_MAX, e_pad - e0)
        superchunks.append((e0, n))
        e0 += n
    return dict(
        chunk_meta=chunk_meta,
        S_packed=S_packed,
        idx_layout=idx_layout,
        e_pad=e_pad,
        superchunks=superchunks,
    )


def _patch_drain(tile, mybir):
    # the TileContext tail drain can carry more sem waits than one
    # instruction supports; split them across extra drains
    from concourse.vector_clock import ScopedClock

    def _drain_and_barrier(self, tick_clock, wait_clock):
        nc = self.nc
        drain_inst = nc.sync.drain()
        wait_clock.add_sem_waits(
            drain_inst.ins, ScopedClock({None: tick_clock.global_clock})
        )
        si = drain_inst.ins.sync_info
        waits = list(si.on_wait) if (si is not None and si.on_wait) else []
        if len(waits) > 1:
            drain_inst.ins.sync_info = mybir.SyncInfo(
                on_wait=waits[:1], on_update=list(si.on_update or [])
            )
            for w in waits[1:]:
                d2 = nc.sync.drain()
                d2.ins.sync_info = mybir.SyncInfo(on_wait=[w], on_update=[])
        nc.all_engine_barrier()
        assert self.sems is not None
        popped = nc._tile_sem_poison_stack.pop()
        assert popped is self._sem_poison
        nc.clear_and_free_semaphores(list(self.sems.allocated().values()))
        nc.all_engine_barrier()

    tile.TileContext._drain_and_barrier = _drain_and_barrier


def _build(prep):
    import concourse.bacc as bacc
    import concourse.mybir as mybir
    import concourse.tile as tile

    _patch_drain(tile, mybir)

    meta = prep["chunk_meta"]
    e_pad = prep["e_pad"]
    c_total = prep["S_packed"].shape[1]
    f32 = mybir.dt.float32

    nc = bacc.Bacc(
        "TRN2",
        target_bir_lowering=False,
        debug=False,
        num_devices=B,
        num_swdge_queues=NQUEUES,
    )
    xT = nc.dram_tensor("xT", (D, N), f32, kind="ExternalInput").ap()
    w_in = nc.dram_tensor("w_in", (D, 3 * D), f32, kind="ExternalInput").ap()
    sp = nc.dram_tensor("sp", (CHUNK, c_total), f32, kind="ExternalInput").ap()
    gidx = nc.dram_tensor(
        "gidx", (128, e_pad // 16), mybir.dt.int16, kind="ExternalInput"
    ).ap()
    hp = [
        nc.dram_tensor(f"hp{i}", (N, D), f32, kind="Internal").ap() for i in range(2)
    ]
    outT = nc.dram_tensor("outT", (D, N), f32, kind="ExternalOutput").ap()

    n_row_tiles = (N + 127) // 128

    with tile.TileContext(nc) as tc:
        with (
            tc.tile_pool(name="persist", bufs=1) as persist,
            tc.tile_pool(name="mg", bufs=4) as mg_pool,
            tc.tile_pool(name="hrm", bufs=4) as hrm_pool,
            tc.tile_pool(name="ostage", bufs=2) as out_pool,
            tc.tile_pool(name="psd", bufs=2, space="PSUM") as dense_psum,
            tc.tile_pool(name="psg", bufs=3, space="PSUM") as spmm_psum,
        ):
            h_fm = persist.tile([D, N], f32)
            nc.sync.dma_start(h_fm[:], xT[:])
            z_sb = persist.tile([CHUNK, D], f32)
            nc.gpsimd.memset(z_sb[:], 0.0)
            w_sb = persist.tile([D, 3 * D], f32)
            nc.sync.dma_start(w_sb[:], w_in[:])
            s_sb = persist.tile([CHUNK, c_total], f32)
            nc.sync.dma_start(s_sb[:], sp[:])
            gidx_sb = persist.tile([128, e_pad // 16], mybir.dt.int16)
            nc.sync.dma_start(gidx_sb[:], gidx[:])

            for layer in range(3):
                src = hp[layer % 2]
                # dense: H' = relu(H @ W.T) row-major -> DRAM gather source
                for nt in range(n_row_tiles):
                    n0 = nt * 128
                    rr = min(128, N - n0)
                    psd = dense_psum.tile([128, D], f32)
                    nc.tensor.matmul(
                        psd[:rr, :],
                        lhsT=h_fm[:, n0 : n0 + rr],
                        rhs=w_sb[:, layer * D : (layer + 1) * D],
                        start=True,
                        stop=True,
                    )
                    hrm = hrm_pool.tile([128, D], f32)
                    nc.scalar.activation(
                        hrm[:rr, :], psd[:rr, :], mybir.ActivationFunctionType.Relu
                    )
                    nc.sync.dma_start(src[n0 : n0 + rr, :], hrm[:rr, :])

                # spmm: gather messages, segmented-sum matmuls into PSUM
                ci = 0
                psg = None
                for si_, (e0, ne) in enumerate(prep["superchunks"]):
                    nch = ne // CHUNK
                    mg = mg_pool.tile([128, SC_MAX // CHUNK, D], f32)
                    nc.gpsimd.dma_gather(
                        mg[:, :nch, :],
                        src[:],
                        gidx_sb[:, e0 // 16 : (e0 + ne) // 16],
                        num_idxs=ne,
                        num_idxs_reg=ne,
                        elem_size=D,
                        queue_num=si_ % NQUEUES,
                        single_packet=False,
                    )
                    for j in range(nch):
                        g, r_lo, R, s_off, first, last = meta[ci]
                        ci += 1
                        if first:
                            psg = spmm_psum.tile([D, GROUP], f32)
                            # opening zero-matmul: clears the bank's
                            # has_written and writes zeros so the chunk
                            # matmuls can accumulate with overlapping views
                            nc.tensor.matmul(
                                psg[:, :],
                                lhsT=z_sb[:, :],
                                rhs=s_sb[:, 0:GROUP],
                                start=True,
                                stop=False,
                            )
                        nc.tensor.matmul(
                            psg[:, r_lo : r_lo + R],
                            lhsT=mg[:, j, :],
                            rhs=s_sb[:, s_off : s_off + R],
                            start=False,
                            stop=last,
                        )
                        if last:
                            lo = g * GROUP
                            gs = min(GROUP, N - lo)
                            if layer < 2:
                                nc.vector.tensor_copy(
                                    h_fm[:, lo : lo + gs], psg[:, :gs]
                                )
                            else:
                                ot = out_pool.tile([D, GROUP], f32)
                                nc.vector.tensor_copy(ot[:, :gs], psg[:, :gs])
                                nc.sync.dma_start(outT[:, lo : lo + gs], ot[:, :gs])

    nc.compile()
    return nc


def _get_compiled(adj_vals, adj_rows, adj_cols):
    key = hash((adj_rows.tobytes(), adj_cols.tobytes(), adj_vals.tobytes()))
    if key not in _CACHE:
        prep = _preprocess(adj_vals, adj_rows, adj_cols)
        nc = _build(prep)
        _CACHE[key] = (prep, nc)
    return _CACHE[key]


def _in_maps(prep, x, W0, W1, W2):
    w_in = np.ascontiguousarray(
        np.concatenate([W0.T, W1.T, W2.T], axis=1), dtype=np.float32
    )
    return [
        {
            "xT": np.ascontiguousarray(x[b].T, dtype=np.float32),
            "w_in": w_in,
            "sp": prep["S_packed"],
            "gidx": prep["idx_layout"],
        }
        for b in range(B)
    ]


def kernel(x, W0, W1, W2, adj_vals, adj_rows, adj_cols):
    from concourse import bass_utils

    x = np.asarray(x, dtype=np.float32)
    prep, nc = _get_compiled(
        np.asarray(adj_vals, dtype=np.float32),
        np.asarray(adj_rows, dtype=np.int32),
        np.asarray(adj_cols, dtype=np.int32),
    )
    in_maps = _in_maps(
        prep,
        x,
        np.asarray(W0, dtype=np.float32),
        np.asarray(W1, dtype=np.float32),
        np.asarray(W2, dtype=np.float32),
    )
    res = bass_utils.run_bass_kernel_spmd(nc, in_maps, core_ids=list(range(B)))
    out = np.stack([res.results[c]["outT"] for c in range(B)])  # [B, D, N]
    return np.ascontiguousarray(np.transpose(out, (0, 2, 1)))
